# revision 15
# baseline (speedup 1.0000x reference)
"""Trainium2 Bass kernel for depthwise-multiplier conv + ReLU + per-out-channel
1x1 combine (nn_Comb_70016556859799).

Math (reference):
  out[b,o,p,q] = bc[o] + sum_i Wc[o,i] * relu( sum_{dy,dx} Wf[o,i,dy,dx]*x[b,i,p+dy,q+dx] + bf[o,i] )

Sharding: 8 cores = (batch b in 0..3) x (H half in 0..1). Each core computes
out[b, :, 63*h : 63*h+63, :] from x[b, :, 63*h : 63*h+66, :].

Per-core dataflow (K=126 im2col packing):
  - input: padded x slice [70ch, 67*128] bf16 in HBM. Five resident SBUF
    tiles [126, 8192]: partition (14*t + cl)*9 + tap holds channel 14t+cl
    shifted by tap offset (dy*128+dx) -- the 9x im2col replication is done
    by the load DMA itself via overlapping 16KB descriptors.
  - conv: for each channel pair p (2 channels x 64 outs = 128 planes) and
    512-pixel chunk: one matmul z[128,512] = w2[:,p,:]^T(126x128) @ xs. The
    lhsT is zero outside the pair's 18 rows, so each column cycle yields 128
    conv outputs (the PE column-stream optimum).
  - relu+bias -> bf16 SBUF, split across Act/DVE/Pool engines (13/11/8).
  - combine: po[64*ch..][512] += ds[:,p,:]^T(128x64 stacked-diag Wc) @ r,
    PSUM-accumulated over the 32 pairs; two 4-row chunks share one PSUM bank
    (partitions 0:64 / 64:128).
  - evac: += bc on ScalarE, one 2-chunk output DMA per chunk-group.
PE emission is software-pipelined (combine lags conv by 2 steps); PSUM uses
all 8 banks (3x2 z double-buffers + 2x1 po).
"""
import numpy as np
import ml_dtypes

import concourse.bass as bass
import concourse.mybir as mybir
from concourse import tile
from concourse.bass_utils import run_bass_kernel_spmd

BF16 = mybir.dt.bfloat16
F32 = mybir.dt.float32
npbf16 = ml_dtypes.bfloat16

B, FIN, FOUT, KK, H, W = 4, 64, 64, 3, 128, 128
HO, WO = H - KK + 1, W - KK + 1          # 126, 126
RPC = HO // 2                             # 63 output rows per core
XC = 128                                  # padded row width
NPAIR = FIN // 2                          # 32 channel pairs
NTILE = 5                                 # xs tiles of 14 channels each
CPT = 14                                  # channels per tile
PPT = 7                                   # pairs per tile
NCG = 8                                   # chunk groups (2 chunks each)
CHUNK = 512                               # pixels per chunk (4 rows x 128)
XROW = 67                                 # padded input rows per core
PIPE = 3                                  # combine lags conv by PIPE steps

# relu engine schedule: Pool/GPSIMD cannot read PSUM on TRN2, so split
# across Act (.833 ns/col) and DVE (1.042 ns/col): 17/15, interleaved.
def _relu_order():
    counts = {"a": 17, "d": 15}
    acc = {e: 0.0 for e in counts}
    order = []
    for _ in range(NPAIR):
        for e in counts:
            acc[e] += counts[e] / float(NPAIR)
        pick = max(acc, key=lambda e: (acc[e], e))
        acc[pick] -= 1.0
        order.append(pick)
    return order

RELU_ORDER = _relu_order()


def _hoist_extra_waits(nc):
    """Walrus supports only one sync-wait command per instruction. Tile can
    emit several (multiple producer procs). Hoist all but the last wait onto
    fresh same-engine NoOp instructions placed immediately before -- the waits
    still execute on the same engine sequencer in the same order, so the
    synchronization semantics are unchanged."""
    import copy
    n_hoist = 0
    for blk in nc.m.functions[0].blocks:
        newinsts = []
        for inst in blk.instructions:
            si = getattr(inst, "sync_info", None)
            ow = list(si.on_wait) if si is not None and si.on_wait else []
            if len(ow) > 1:
                for wi, w in enumerate(ow[:-1]):
                    nop = mybir.InstNoOp(
                        name=f"{inst.name}_hw{wi}",
                        text_hint="hoisted_wait",
                        bass_nofuse=True,
                    )
                    nop.engine = inst.engine
                    nsi = copy.deepcopy(si)
                    nsi.on_wait = [w]
                    if getattr(nsi, "on_update", None):
                        nsi.on_update = []
                    nop.sync_info = nsi
                    newinsts.append(nop)
                    n_hoist += 1
                si.on_wait = [ow[-1]]
            newinsts.append(inst)
        blk.instructions = newinsts
    return n_hoist


def _build(hoist=True):
    nc = bass.Bass()
    xs_d = nc.declare_dram_parameter("xs", [CPT * NTILE * KK, XROW * XC], BF16,
                                     isOutput=False)
    w2_d = nc.declare_dram_parameter("w2", [126, NPAIR, 128], BF16,
                                     isOutput=False)
    ds_d = nc.declare_dram_parameter("ds", [128, NPAIR, FOUT], BF16,
                                     isOutput=False)
    bfp_d = nc.declare_dram_parameter("bfp", [128, NPAIR], F32, isOutput=False)
    bc2_d = nc.declare_dram_parameter("bc2", [128, 1], F32, isOutput=False)
    out_d = nc.declare_dram_parameter("out", [FOUT, 64, XC], F32, isOutput=True)

    AF = mybir.ActivationFunctionType
    ALU = mybir.AluOpType

    with tile.TileContext(nc) as tc:
        with (
            tc.tile_pool(name="wpool", bufs=1) as wpool,
            tc.tile_pool(name="xpool", bufs=1) as xpool,
            tc.tile_pool(name="rpool", bufs=4) as rpool,
            tc.tile_pool(name="opool", bufs=2) as opool,
            tc.tile_pool(name="psz", bufs=3, space=bass.MemorySpace.PSUM) as psz,
            tc.tile_pool(name="pso", bufs=2, space=bass.MemorySpace.PSUM) as pso,
        ):
            # startup DMA schedule: just-in-time ordering so the first conv
            # can fire ~2.5us in. w2/ds split at pair 8; xs tiles split into a
            # cg0 head + cg1-7 tail.
            bc2_t = wpool.tile([128, 1], F32, tag="bc2")
            nc.sync.dma_start(bc2_t[:], bc2_d[:])
            bfp_t = wpool.tile([128, NPAIR], F32, tag="bfp")
            nc.sync.dma_start(bfp_t[:], bfp_d[:])
            w2_t = wpool.tile([126, NPAIR, 128], BF16, tag="w2")
            ds_t = wpool.tile([128, NPAIR, FOUT], BF16, tag="ds")
            nc.sync.dma_start(w2_t[:, 0:8, :], w2_d[:, 0:8, :])
            xs_t = []
            for t in range(NTILE):
                xt = xpool.tile([126, NCG, 2, CHUNK], BF16, tag=f"xs{t}")
                xs_t.append(xt)

            def xs_src(t, lo, hi):
                return bass.AP(
                    xs_d,
                    CPT * KK * t * XROW * XC + lo * 2 * CHUNK,
                    [[XROW * XC, CPT * KK], [1, KK], [1, (hi - lo) * 2 * CHUNK]],
                )

            nc.sync.dma_start(xs_t[0][:, 0, :, :], xs_src(0, 0, 1))
            nc.sync.dma_start(ds_t[:, 0:8, :], ds_d[:, 0:8, :])
            nc.sync.dma_start(xs_t[1][:, 0, :, :], xs_src(1, 0, 1))
            nc.sync.dma_start(w2_t[:, 8:NPAIR, :], w2_d[:, 8:NPAIR, :])
            for t in range(2, NTILE):
                nc.sync.dma_start(xs_t[t][:, 0, :, :], xs_src(t, 0, 1))
            nc.sync.dma_start(ds_t[:, 8:NPAIR, :], ds_d[:, 8:NPAIR, :])
            for t in range(NTILE):
                nc.sync.dma_start(xs_t[t][:, 1:, :, :], xs_src(t, 1, NCG))

            # PE ramp warmers: a memset scratch feeds dummy matmuls that keep
            # the PE busy (and p-state ramping) while the weight DMAs land.
            scr_m = wpool.tile([128, 640], BF16, tag="scr_m")
            nc.gpsimd.memset(scr_m[:], 0.0)
            dummy = psz.tile([128, 2, CHUNK], F32, tag="z", name="zdummy")
            for wi in range(6):
                nc.tensor.matmul(
                    dummy[:, wi % 2, :], scr_m[:, 0:128], scr_m[:, 128:640],
                    start=True, stop=True,
                )
            # engine warmups: observe the weight-DMA semaphores once via tiny
            # dummy ops so real instructions never carry those waits.
            nc.tensor.matmul(
                dummy[0:64, 0, 0:64], w2_t[0:18, 0, 0:64], w2_t[0:18, 1, 0:64],
                start=True, stop=True,
            )
            nc.tensor.matmul(
                dummy[0:64, 1, 0:64], ds_t[:, 0, :], ds_t[:, 1, 0:64],
                start=True, stop=True,
            )
            scr_a = wpool.tile([128, 1], F32, tag="scr_a")
            nc.scalar.activation(
                scr_a[:], bc2_t[:], AF.Relu, bias=bfp_t[:, 0:1],
            )
            scr_d = wpool.tile([128, 1], F32, tag="scr_d")
            nc.vector.tensor_scalar(
                scr_d[:], bc2_t[:], bfp_t[:, 0:1], None, ALU.add
            )

            # main pipeline over flat steps s = (cg, p). The last chunk
            # (cg 7, ch 1) holds only 3 valid rows -> 384 columns.
            def ncols(cg, ch):
                return 384 if (cg == NCG - 1 and ch == 1) else CHUNK

            pending = []          # (cg, p, r_tile)
            po = [None, None]     # rotating po tiles by cg parity

            def drain_one():
                cg, p, r_t = pending.pop(0)
                if p == 0:
                    po[cg % 2] = pso.tile([128, CHUNK], F32, tag="po",
                                          name=f"po{cg}")
                pot = po[cg % 2]
                for ch in range(2):
                    n = ncols(cg, ch)
                    nc.tensor.matmul(
                        pot[64 * ch: 64 * ch + 64, 0:n],
                        ds_t[:, p, :],
                        r_t[:, ch, 0:n],
                        start=(p == 0),
                        stop=(p == NPAIR - 1),
                        tile_position=(0, 64 * ch),
                        skip_group_check=True,
                    )
                if p == NPAIR - 1:
                    ob = opool.tile([128, CHUNK], F32, tag="ob", name=f"ob{cg}")
                    if cg < NCG - 1:
                        nc.scalar.activation(
                            ob[:], pot[:], AF.Identity, bias=bc2_t[:, 0:1],
                        )
                    else:
                        nc.scalar.activation(
                            ob[0:64, :], pot[0:64, :], AF.Identity,
                            bias=bc2_t[0:64, 0:1],
                        )
                        nc.scalar.activation(
                            ob[64:128, 0:384], pot[64:128, 0:384], AF.Identity,
                            bias=bc2_t[64:128, 0:1],
                        )
                    if cg < NCG - 1:
                        dst = bass.AP(
                            out_d, 8 * cg * XC,
                            [[4 * XC, 2], [64 * XC, FOUT], [1, CHUNK]],
                        )
                        nc.sync.dma_start(dst, ob[:])
                    else:
                        dst0 = bass.AP(
                            out_d, 8 * cg * XC,
                            [[64 * XC, FOUT], [1, CHUNK]],
                        )
                        nc.sync.dma_start(dst0, ob[0:64, :])
                        dst1 = bass.AP(
                            out_d, (8 * cg + 4) * XC,
                            [[64 * XC, FOUT], [1, 384]],
                        )
                        nc.sync.dma_start(dst1, ob[64:128, 0:384])

            for cg in range(NCG):
                for p in range(NPAIR):
                    t = p // PPT
                    z_t = psz.tile([128, 2, CHUNK], F32, tag="z")
                    for ch in range(2):
                        nc.tensor.matmul(
                            z_t[:, ch, :],
                            w2_t[:, p, :],
                            xs_t[t][:, cg, ch, :],
                            start=True, stop=True,
                        )
                    r_t = rpool.tile([128, 2, CHUNK], BF16, tag="r")
                    if RELU_ORDER[p] == "a":
                        nc.scalar.activation(
                            r_t[:], z_t[:], AF.Relu, bias=bfp_t[:, p: p + 1],
                        )
                    else:
                        nc.vector.tensor_scalar(
                            r_t[:], z_t[:], bfp_t[:, p: p + 1], 0.0,
                            ALU.add, ALU.max,
                        )
                    pending.append((cg, p, r_t))
                    if len(pending) > PIPE:
                        drain_one()
            while pending:
                drain_one()
    if hoist:
        _hoist_extra_waits(nc)
    return nc


_NC = None


def _get_nc():
    global _NC
    if _NC is None:
        _NC = _build()
    return _NC


def _pack_weights(Wf, bf, Wc, bc):
    # conv lhsT: [126, 32 pairs, 128]; pair p covers channels (2p, 2p+1),
    # nonzero rows 18*(p%7) .. +18 (matching its xs tile partitions),
    # col j = 64*jc + o -> Wf[o, 2p+jc, dy, dx] at row offset 9*jc + 3*dy+dx.
    w2 = np.zeros((126, NPAIR, 128), np.float32)
    Wf_flat = Wf.reshape(FOUT, FIN, 9)  # [o, c, tap]
    for p in range(NPAIR):
        i = p % PPT
        for jc in range(2):
            c = 2 * p + jc
            # rows 18i+9jc+tap, cols 64jc+o
            w2[18 * i + 9 * jc: 18 * i + 9 * jc + 9, p, 64 * jc: 64 * jc + 64] = (
                Wf_flat[:, c, :].T
            )
    # combine lhsT: stacked-diagonal Wc pairs [128, 32, 64]
    ds = np.zeros((128, NPAIR, FOUT), np.float32)
    eye = np.eye(FOUT, dtype=np.float32)
    for p in range(NPAIR):
        ds[0:64, p, :] = eye * Wc[:, 2 * p][None, :]
        ds[64:128, p, :] = eye * Wc[:, 2 * p + 1][None, :]
    # conv bias per plane: [128, 32]
    bfp = np.zeros((128, NPAIR), np.float32)
    for p in range(NPAIR):
        bfp[0:64, p] = bf[:, 2 * p]
        bfp[64:128, p] = bf[:, 2 * p + 1]
    bc2 = np.tile(bc.reshape(FOUT, 1), (2, 1)).astype(np.float32)
    return {
        "w2": w2.astype(npbf16),
        "ds": ds.astype(npbf16),
        "bfp": bfp,
        "bc2": bc2,
    }


def _im2col(x, b, h):
    """Padded per-core x slice [210, 67*128] bf16: row 3c+dy holds channel c
    (of 70 = 64 real + 6 zero-pad) shifted by dy rows. The dx shift is done
    by the load DMA's overlapping descriptors."""
    xp = np.zeros((CPT * NTILE, XROW * XC), np.float32)
    r0 = RPC * h
    nrows = min(H - r0, XROW)
    xp[0:FIN, 0: nrows * XC] = x[b, :, r0: r0 + nrows, :].reshape(FIN, -1)
    out = np.zeros((CPT * NTILE * KK, XROW * XC), np.float32)
    for dy in range(KK):
        ncols = XROW * XC - dy * XC
        out[dy::KK, 0:ncols] = xp[:, dy * XC:]
    return np.ascontiguousarray(out).astype(npbf16)


def _run(x, Wf, bf, Wc, bc, **spmd_kwargs):
    shared = _pack_weights(Wf, bf, Wc, bc)
    in_maps = []
    for core in range(8):
        b, h = core // 2, core % 2
        m = dict(shared)
        m["xs"] = _im2col(x, b, h)
        in_maps.append(m)
    res = run_bass_kernel_spmd(_get_nc(), in_maps, list(range(8)), **spmd_kwargs)
    out = np.empty((B, FOUT, HO, WO), np.float32)
    for core in range(8):
        b, h = core // 2, core % 2
        out[b, :, RPC * h: RPC * h + RPC, :] = np.asarray(
            res.results[core]["out"], np.float32
        )[:, 0:RPC, 0:WO]
    return out, res


def kernel(x, Wf, bf, Wc, bc):
    x = np.asarray(x, np.float32)
    out, _ = _run(
        x,
        np.asarray(Wf, np.float32),
        np.asarray(bf, np.float32),
        np.asarray(Wc, np.float32),
        np.asarray(bc, np.float32),
    )
    return out
